# revision 15
# baseline (speedup 1.0000x reference)
"""2-layer GCN (PyG GCNConv x2 + leaky_relu) on 8 Trainium2 NeuronCores.

v4 strategy (pair-packed gather + segment-reduce):
  - Nodes ranked by degree, dealt round-robin across cores (rank k ->
    core k%8, local pos k//8): every core's tile t holds nodes of nearly
    identical degree, so slot capacities are tight and SPMD-uniform.
  - The node table is pair-packed: DRAM row m (256B) holds features of
    gather-rows 2m and 2m+1 (64 bf16 each). dma_gather with idx=row//2
    (always < 25088, int16-safe) lands pairs feature-major: partitions
    0..63 = even row, 64..127 = odd row. Per dst, in-edges (+ self loop)
    are split by src-row parity into even/odd slot grids; one gather per
    half per tile-group, one DVE tensor_reduce per half writing disjoint
    partition halves of a single [128, nd] accumulator.
  - The even/odd halves are merged by the tail matmul itself: W1 is
    duplicated across partitions 0..63/64..127 so the K=128 contraction
    sums both halves (layer 2 uses a stacked-identity matmul instead).
  - Tail per group (<=4 tiles, 512 dst columns/instruction): W1 matmul +
    rank-1 b1*dinv into PSUM, ACT lrelu, W2 matmul, DVE dis^2 scale,
    per-tile PE transpose, ACT copy, one strided store. Layer-2 table is
    built by a 64-col AllGather of the per-core part buffers (the pair
    view is just an AP reshape - no expand pass).
  - Group sizes adapt to per-tile slot caps so the worst gather buffers
    stay small enough for bufs=3 buffering (deep DMA pipeline).

Self-contained: hardcodes shapes; compiles on first call keyed by edge hash.
"""

import os
import hashlib
import sys

import numpy as np

sys.path.insert(0, "/opt/trn_rl_repo")

# ---- problem constants ----
N, E = 50000, 800000
DIN, DH, DOUT = 64, 128, 64
P_CORES = 8
NP = N // P_CORES            # 6250 real nodes per core
NT = 49                      # dst tiles per core
NPP = NT * 128               # 6272 padded rows per core
NROWS = P_CORES * NPP        # 50176 table rows
NPAIR = NROWS // 2           # 25088 pair rows
GSZ = 4                      # max tiles per group
CAP_SLOTS = 7168             # max slots per group-half buffer (14KB/part)
NEG_SLOPE = 0.01


def _prep(edge_index: np.ndarray):
    src = np.asarray(edge_index[0], dtype=np.int64)
    dst = np.asarray(edge_index[1], dtype=np.int64)

    deg = (np.bincount(dst, minlength=N) + 1).astype(np.float32)
    dis = (1.0 / np.sqrt(deg)).astype(np.float32)

    # degree-descending rank -> (core, local pos)
    rank_of = np.argsort(-deg, kind="stable")      # rank -> orig node
    newpos = np.empty(N, dtype=np.int64)           # orig -> rank
    newpos[rank_of] = np.arange(N)
    core_of = newpos % P_CORES
    loc_of = newpos // P_CORES
    grow_of = core_of * NPP + loc_of               # orig -> gather row

    # edge slot lists (self loop appended for every node)
    s_all = np.concatenate([src, np.arange(N, dtype=np.int64)])
    d_all = np.concatenate([dst, np.arange(N, dtype=np.int64)])
    gs = grow_of[s_all]
    dcore = core_of[d_all]
    dloc = loc_of[d_all]
    is_ev = (gs % 2) == 0

    key = ((dcore * NPP + dloc) * 2 + (~is_ev).astype(np.int64))
    order = np.argsort(key, kind="stable")
    ks = key[order]
    first = np.ones(len(ks), dtype=bool)
    first[1:] = ks[1:] != ks[:-1]
    starts = np.flatnonzero(first)
    run_id = np.cumsum(first) - 1
    rnk = np.arange(len(ks)) - starts[run_id]

    gso = gs[order]
    dco = dcore[order]
    dlo = dloc[order]
    evo = is_ev[order]

    cnt_ev = np.zeros((P_CORES, NPP), dtype=np.int64)
    cnt_od = np.zeros((P_CORES, NPP), dtype=np.int64)
    np.add.at(cnt_ev, (dcore[is_ev], dloc[is_ev]), 1)
    np.add.at(cnt_od, (dcore[~is_ev], dloc[~is_ev]), 1)

    # per-tile capacities (max across cores)
    Rt_ev = np.maximum(1, cnt_ev.reshape(P_CORES, NT, 128).max(axis=(0, 2)))
    Rt_od = np.maximum(1, cnt_od.reshape(P_CORES, NT, 128).max(axis=(0, 2)))

    # adaptive grouping: <= GSZ tiles and <= CAP_SLOTS per half
    groups = []                                    # list of (t0, nt)
    t = 0
    while t < NT:
        nt = 1
        while (t + nt < NT and nt < GSZ
               and (nt + 1) * 128 * max(Rt_ev[t:t + nt + 1]) <= CAP_SLOTS
               and (nt + 1) * 128 * max(Rt_od[t:t + nt + 1]) <= CAP_SLOTS):
            nt += 1
        groups.append((t, nt))
        t += nt
    R_ev = np.array([max(Rt_ev[t0:t0 + nt]) for t0, nt in groups])
    R_od = np.array([max(Rt_od[t0:t0 + nt]) for t0, nt in groups])

    n_ev = [int(nt * 128 * R_ev[g]) for g, (t0, nt) in enumerate(groups)]
    n_od = [int(nt * 128 * R_od[g]) for g, (t0, nt) in enumerate(groups)]
    off_ev = np.concatenate([[0], np.cumsum(n_ev)]).astype(np.int64)
    off_od = np.concatenate([[0], np.cumsum(n_od)]).astype(np.int64)
    tot_ev, tot_od = int(off_ev[-1]), int(off_od[-1])

    DUMMY = NP // 2        # pair row 3125 = rows (6250, 6251), core0 pads
    idx_ev = np.full((P_CORES, tot_ev), DUMMY, dtype=np.int64)
    idx_od = np.full((P_CORES, tot_od), DUMMY, dtype=np.int64)

    # map each loc to its group and position within the group
    g_of_tile = np.empty(NT, dtype=np.int64)
    qoff_of_tile = np.empty(NT, dtype=np.int64)    # (q*128) offset in group
    for g, (t0, nt) in enumerate(groups):
        g_of_tile[t0:t0 + nt] = g
        qoff_of_tile[t0:t0 + nt] = np.arange(nt) * 128
    tile_of = dlo // 128
    gg = g_of_tile[tile_of]
    qp = qoff_of_tile[tile_of] + (dlo % 128)

    sel = evo
    j = off_ev[gg[sel]] + qp[sel] * R_ev[gg[sel]] + rnk[sel]
    idx_ev[dco[sel], j] = gso[sel] // 2
    sel = ~evo
    j = off_od[gg[sel]] + qp[sel] * R_od[gg[sel]] + rnk[sel]
    idx_od[dco[sel], j] = gso[sel] // 2

    def wrap16(a, tot):
        cols = tot // 16
        t = np.zeros((P_CORES, 128, cols), dtype=np.int16)
        v = a.astype(np.int16).reshape(P_CORES, cols, 16)
        for rs in range(0, 128, 16):
            t[:, rs:rs + 16, :] = v.transpose(0, 2, 1)
        return t

    assert tot_ev % 16 == 0 and tot_od % 16 == 0
    idx_ev_t = wrap16(idx_ev, tot_ev)
    idx_od_t = wrap16(idx_od, tot_od)

    dis_loc = np.zeros((P_CORES, NPP), dtype=np.float32)
    for c in range(P_CORES):
        n_ids = rank_of[c::P_CORES]
        dis_loc[c, :len(n_ids)] = dis[n_ids]
    disrow = np.broadcast_to(dis_loc[:, None, :], (P_CORES, 64, NPP)).copy()
    disrow2 = (disrow * disrow).copy()
    dinv = np.where(dis_loc > 0, 1.0 / np.maximum(dis_loc, 1e-9), 0.0)
    dinv = dinv.reshape(P_CORES, 1, NPP).astype(np.float32)

    return dict(dis=dis, rank_of=rank_of,
                groups=groups, R_ev=R_ev, R_od=R_od,
                off_ev=off_ev, off_od=off_od, tot_ev=tot_ev, tot_od=tot_od,
                idx_ev=idx_ev_t, idx_od=idx_od_t,
                disrow=disrow, disrow2=disrow2, dinv=dinv)


# ---------------------------------------------------------------------------
# Bass kernel
# ---------------------------------------------------------------------------

def _build_nc(prep):
    import concourse.bass as bass
    import concourse.bacc as bacc
    import concourse.tile as tile
    from concourse import mybir

    f32 = mybir.dt.float32
    bf16 = mybir.dt.bfloat16
    i16 = mybir.dt.int16
    AF = mybir.ActivationFunctionType
    ALU = mybir.AluOpType
    AX = mybir.AxisListType
    ds = bass.ds

    groups = prep["groups"]
    R_ev, R_od = prep["R_ev"], prep["R_od"]
    off_ev, off_od = prep["off_ev"], prep["off_od"]
    tot_ev, tot_od = prep["tot_ev"], prep["tot_od"]

    nc = bacc.Bacc(
        "TRN2", target_bir_lowering=False, debug=False,
        enable_asserts=False, num_devices=P_CORES,
    )

    xt_d = nc.dram_tensor("xt", [NPAIR, 128], bf16, kind="ExternalInput")
    ixev_d = nc.dram_tensor("ixev", [128, tot_ev // 16], i16,
                            kind="ExternalInput")
    ixod_d = nc.dram_tensor("ixod", [128, tot_od // 16], i16,
                            kind="ExternalInput")
    disr_d = nc.dram_tensor("disr", [64, NPP], f32, kind="ExternalInput")
    disr2_d = nc.dram_tensor("disr2", [64, NPP], bf16, kind="ExternalInput")
    dinv_d = nc.dram_tensor("dinv", [1, NPP], f32, kind="ExternalInput")
    w1_d = nc.dram_tensor("w1d", [DH, DH], f32, kind="ExternalInput")
    w2_d = nc.dram_tensor("w2b", [DH, DOUT], bf16, kind="ExternalInput")
    b1_d = nc.dram_tensor("b1r", [1, DH], f32, kind="ExternalInput")
    b2_d = nc.dram_tensor("b2c", [64, 1], f32, kind="ExternalInput")
    identb_d = nc.dram_tensor("identb", [64, 64], bf16, kind="ExternalInput")
    identf_d = nc.dram_tensor("identf", [64, 64], f32, kind="ExternalInput")
    ident2_d = nc.dram_tensor("ident2", [DH, 64], f32, kind="ExternalInput")
    out_d = nc.dram_tensor("outp", [64, NPP], f32, kind="ExternalOutput")

    with tile.TileContext(nc) as tc:
        with (
            tc.tile_pool(name="const", bufs=1) as constp,
            tc.tile_pool(name="gev", bufs=3) as gevp,
            tc.tile_pool(name="god", bufs=3) as godp,
            tc.tile_pool(name="red", bufs=3) as redp,
            tc.tile_pool(name="wk", bufs=2) as work,
            tc.tile_pool(name="stg", bufs=2) as stgp,
            tc.tile_pool(name="px1", bufs=2, space="PSUM") as px1,
            tc.tile_pool(name="pp2", bufs=1, space="PSUM") as pp2,
            tc.tile_pool(name="ptr", bufs=1, space="PSUM") as ptr,
            tc.tile_pool(name="dram", bufs=1, space="DRAM") as dram,
        ):
            ixev_sb = constp.tile([128, tot_ev // 16], i16)
            ixod_sb = constp.tile([128, tot_od // 16], i16)
            disr_sb = constp.tile([64, NPP], f32)
            disr2_sb = constp.tile([64, NPP], bf16)
            dinv_sb = constp.tile([1, NPP], f32)
            w1_sb = constp.tile([DH, DH], f32)
            w2_sb = constp.tile([DH, DOUT], bf16)
            b1_sb = constp.tile([1, DH], f32)
            b2_sb = constp.tile([64, 1], f32)
            identb_sb = constp.tile([64, 64], bf16)
            identf_sb = constp.tile([64, 64], f32)
            ident2_sb = constp.tile([DH, 64], f32)
            for sb, dr in [(ixev_sb, ixev_d), (ixod_sb, ixod_d),
                           (disr_sb, disr_d), (disr2_sb, disr2_d),
                           (dinv_sb, dinv_d), (w1_sb, w1_d), (w2_sb, w2_d),
                           (b1_sb, b1_d), (b2_sb, b2_d),
                           (identb_sb, identb_d), (identf_sb, identf_d),
                           (ident2_sb, ident2_d)]:
                nc.sync.dma_start(sb[:], dr[:])

            abl = os.environ.get("GCN_ABL", "")
            for _rep in range(int(os.environ.get("GCN_REPEAT", "1"))):
                part = dram.tile([NPP, 64], bf16, tag="part", bufs=2)
                table = dram.tile([NROWS, 64], bf16, addr_space="Shared",
                                  tag="table", bufs=2)

                for lidx in range(2):
                    tabv = (xt_d[:, :] if lidx == 0 else
                            table[:, :].rearrange("(n k) f -> n (k f)", k=2))
                    for g, (t0, nt) in enumerate(groups):
                        nd = nt * 128
                        rev, rod = int(R_ev[g]), int(R_od[g])
                        nev, nod = nd * rev, nd * rod
                        gev = gevp.tile([128, nev], bf16, tag="gev")
                        if abl != "nog":
                            nc.gpsimd.dma_gather(
                                gev[:].rearrange("p (one n) -> p one n",
                                                 one=1),
                                tabv,
                                ixev_sb[:, ds(int(off_ev[g]) // 16,
                                              nev // 16)],
                                num_idxs=nev, num_idxs_reg=nev,
                                elem_size=128, transpose=True,
                                single_packet=False)
                        god = godp.tile([128, nod], bf16, tag="god")
                        if abl != "nog":
                            nc.gpsimd.dma_gather(
                                god[:].rearrange("p (one n) -> p one n",
                                                 one=1),
                                tabv,
                                ixod_sb[:, ds(int(off_od[g]) // 16,
                                              nod // 16)],
                                num_idxs=nod, num_idxs_reg=nod,
                                elem_size=128, transpose=True,
                                single_packet=False)
                        r12 = redp.tile([128, nd], f32, tag="r12")
                        nc.vector.tensor_reduce(
                            r12[0:64, :],
                            gev[0:64, :].rearrange("p (d r) -> p d r", r=rev),
                            axis=AX.X, op=ALU.add)
                        nc.vector.tensor_reduce(
                            r12[64:128, :],
                            god[64:128, :].rearrange("p (d r) -> p d r",
                                                     r=rod),
                            axis=AX.X, op=ALU.add)
                        col0 = t0 * 128
                        if abl == "gonly":
                            if lidx == 1:
                                ost = stgp.tile([64, 64], f32, tag="ostg")
                                nc.scalar.activation(
                                    ost[:], r12[0:64, 0:64], AF.Copy,
                                    bias=0.0)
                                nc.sync.dma_start(
                                    out_d[:, ds(col0, 64)], ost[:])
                            continue
                        if lidx == 0:
                            x1p = px1.tile([DH, nd], f32, tag="x1p")
                            nc.tensor.matmul(x1p[:], w1_sb[:], r12[:],
                                             start=True, stop=False)
                            nc.tensor.matmul(
                                x1p[:], b1_sb[:],
                                dinv_sb[:, ds(col0, nd)],
                                start=False, stop=True)
                            x1sb = work.tile([DH, nd], bf16, tag="x1sb")
                            nc.scalar.activation(x1sb[:], x1p[:], AF.Lrelu,
                                                 bias=0.0, alpha=NEG_SLOPE)
                            p2p = pp2.tile([64, nd], f32, tag="p2p")
                            nc.tensor.matmul(p2p[:], w2_sb[:], x1sb[:],
                                             start=True, stop=True)
                            pts = work.tile([64, nd], bf16, tag="pts")
                            nc.vector.tensor_tensor(
                                pts[:], p2p[:], disr2_sb[:, ds(col0, nd)],
                                op=ALU.mult)
                            pT = ptr.tile([128, nt * 64], bf16, tag="pT")
                            for q in range(nt):
                                nc.tensor.transpose(
                                    pT[:, q * 64:(q + 1) * 64],
                                    pts[:, q * 128:(q + 1) * 128],
                                    identb_sb[:])
                            stage = stgp.tile([128, nt * 64], bf16,
                                              tag="stage")
                            nc.scalar.activation(stage[:], pT[:], AF.Copy,
                                                 bias=0.0)
                            nc.sync.dma_start(
                                part[ds(col0, nd), :].rearrange(
                                    "(q p) f -> p q f", p=128),
                                stage[:].rearrange("p (q f) -> p q f", f=64))
                        else:
                            a2p = pp2.tile([64, nd], f32, tag="a2p")
                            nc.tensor.matmul(a2p[:], ident2_sb[:], r12[:],
                                             start=True, stop=True)
                            aggs = work.tile([64, nd], f32, tag="aggs")
                            nc.vector.tensor_tensor(
                                aggs[:], a2p[:], disr_sb[:, ds(col0, nd)],
                                op=ALU.mult)
                            osbT = work.tile([64, nd], f32, tag="osbT")
                            nc.scalar.activation(osbT[:], aggs[:], AF.Lrelu,
                                                 bias=b2_sb[:, 0:1],
                                                 alpha=NEG_SLOPE)
                            nc.sync.dma_start(
                                out_d[:, ds(col0, nd)], osbT[:])
                    if lidx == 0 and abl != "gonly":
                        if os.environ.get("GCN_NOAG", "0") == "1":
                            pass
                        else:
                            nc.gpsimd.collective_compute(
                                "AllGather", mybir.AluOpType.bypass,
                                replica_groups=[list(range(P_CORES))],
                                ins=[part.opt()], outs=[table.opt()],
                            )

    nc.compile()
    return nc


def _make_in_maps(inputs, W1, b1, W2, b2, prep):
    import ml_dtypes
    dis = prep["dis"]
    rank_of = prep["rank_of"]
    x32 = np.asarray(inputs, np.float32) * dis[:, None]   # dis_s * x_s
    xt = np.zeros((NROWS, 64), dtype=np.float32)
    k = np.arange(N)
    rows = (k % P_CORES) * NPP + (k // P_CORES)
    xt[rows, :] = x32[rank_of]
    xt = xt.reshape(NPAIR, 128).astype(ml_dtypes.bfloat16)
    ident = np.eye(64, dtype=np.float32)
    W1np = np.asarray(W1, np.float32)
    w1dup = np.concatenate([W1np, W1np], axis=0)          # [128, 128]
    ident2 = np.concatenate([ident, ident], axis=0)       # [128, 64]
    in_maps = []
    for c in range(P_CORES):
        in_maps.append({
            "xt": xt,
            "ixev": prep["idx_ev"][c],
            "ixod": prep["idx_od"][c],
            "disr": prep["disrow"][c],
            "disr2": prep["disrow2"][c].astype(ml_dtypes.bfloat16),
            "dinv": prep["dinv"][c],
            "w1d": w1dup,
            "w2b": np.asarray(W2, np.float32).astype(ml_dtypes.bfloat16),
            "b1r": np.asarray(b1, np.float32).reshape(1, DH),
            "b2c": np.asarray(b2, np.float32).reshape(64, 1),
            "identb": ident.astype(ml_dtypes.bfloat16),
            "identf": ident,
            "ident2": ident2,
        })
    return in_maps


_CACHE = {}


def kernel(inputs, edge_index, W1, b1, W2, b2, _trace=False, _results_box=None):
    from concourse.bass_utils import run_bass_kernel_spmd

    edge_index = np.asarray(edge_index)
    key = hashlib.sha1(edge_index.tobytes()).hexdigest()
    key += ":r%s:n%s:a%s" % (os.environ.get("GCN_REPEAT", "1"),
                             os.environ.get("GCN_NOAG", "0"),
                             os.environ.get("GCN_ABL", ""))
    if key not in _CACHE:
        prep = _prep(edge_index)
        nc = _build_nc(prep)
        _CACHE[key] = (prep, nc)
    prep, nc = _CACHE[key]
    in_maps = _make_in_maps(inputs, W1, b1, W2, b2, prep)
    res = run_bass_kernel_spmd(
        nc, in_maps, core_ids=list(range(P_CORES)), trace=_trace,
    )
    if _results_box is not None:
        _results_box.append(res)
    outp = np.empty((N, DOUT), dtype=np.float32)
    rank_of = prep["rank_of"]
    for c in range(P_CORES):
        o = res.results[c]["outp"][:, :NP]           # [64, NP] feature-major
        ranks = np.arange(NP) * P_CORES + c
        outp[rank_of[ranks]] = o.T
    return outp
